# revision 15
# baseline (speedup 1.0000x reference)
"""Trainium2 Bass kernel for the pairwise-biased attention layer.

Problem (B=4, N=2048, E=512, H=8, D=64):
    Q,K,V = Linear(x); scores = QK^T/sqrt(D) + MLP(pairwise_feats) per head;
    masked softmax over keys; out = Linear(attn @ V).

Sharding: 8 cores = (batch b in 0..3) x (query half in 0..1). Each core
computes all heads for its 1024 query rows of one batch; outputs are
disjoint, so the host just concatenates (no collectives).

Key device-side structure (per core), v3:
  - scores are built TRANSPOSED: S^T[j, i] = K @ Q^T per head.  Heads are
    processed in PAIRS (2et, 2et+1): K^T/Q^T live head-paired in one
    [128, .] tile (head 2et in partitions 0-63, head 2et+1 in 64-127), so
    the two K=64 score matmuls run CONCURRENTLY in the PE array via row
    tiling (tile_position (0,0) / (64,0)).
  - the pairwise-bias MLP (hidden=16, b1=0) collapses per head to
    bias_h(p) = cneg_h*p + (cpos_h-cneg_h)*relu(p).  Per head one of:
      'F': ONE fused custom-DVE op  X = c0*p + c1*relu(p) + S(psum),
           then grouped ACT exp.
      'D': both terms accumulated into the score PSUM by diagonal-matrix
           matmuls (quadrant-packed 64x64 pairs) on p and relu(p); exp
           reads PSUM directly.  relu(p) is materialized once on device.
  - softmax denominator comes free from a ones column appended to V
    (attn@V matmul M=65); z row is copied to SBUF by ScalarE,
    reciprocal_approx_fast + gpsimd broadcast + one DVE multiply
    normalize the output.
  - key padding handled generically: only ceil(n_valid/128) key tiles are
    computed at all; any masked keys inside that range get -60000 added
    via the exp's per-partition bias vector (slow path; the standard mask
    is handled entirely by the fast path at zero cost).
"""

import sys

sys.path.insert(0, "/opt/trn_rl_repo")

import numpy as np
import ml_dtypes

import concourse.bass as bass
import concourse.tile as tile
from concourse import bacc, mybir
from concourse import dve_ops
from concourse.dve_spec import Spec, Src0, Src1, C0, C1, relu as dve_relu
from concourse.dve_spec import lower as dve_lower, _has_src1
from concourse.dve_uop import DveOpSpec
from concourse.bass_utils import run_bass_kernel_spmd

B, N, E, H, D = 4, 2048, 512, 8, 64
NCORES = 8
IH = N // 2  # query rows per core (1024)
BF16 = mybir.dt.bfloat16
F32 = mybir.dt.float32
NEG_BIG = -60000.0

# Per-head bias route: 'F' = fused custom-DVE op, 'D' = PE diag-matmuls.
ROUTES = "DFFFDFFF"
EXP_GROUP = 3  # j-tiles per grouped exp instruction (fast path)


def _make_bias_add_op():
    """Register out = c0*in0 + c1*relu(in0) + in1 as a custom DVE op."""
    name = "BIAS_ADD_PRELU_ANT"
    if name in dve_ops._SUB_OPCODE_FOR_NAME:
        return next(op for op in dve_ops.OPS if op.name == name)
    spec = Spec(
        body=(Src0 * C0 + dve_relu(Src0) * C1) + Src1,
        reference=lambda in0, in1, s0, s1, imm2: (
            in0.astype(np.float32) * s0
            + np.maximum(np.nan_to_num(in0.astype(np.float32), nan=0.0), 0.0)
            * s1
            + in1
        ),
    )
    row = max(dve_ops._SUB_OPCODE_FOR_NAME.values()) + 1
    assert row < 0x20
    shas = {}
    for ver in ("v3", "v4"):
        tmp = DveOpSpec(name=name, opcode=row, uops=dve_lower(spec, ver=ver),
                        rd1_en=_has_src1(spec))
        shas[ver] = tmp.sha(ver)
    op = dve_ops.DveOp(name, spec, subdim=False, uops_sha=shas)
    dve_ops.OPS.append(op)
    dve_ops.CUSTOM_DVE_SPECS[name] = spec
    dve_ops._SUB_OPCODE_FOR_NAME[name] = row
    return op


BIAS_ADD_OP = _make_bias_add_op()


def _idm_terms(routes):
    """(head, which) list for the diag-matmul terms, in idm tensor order.
    which: 'raw' -> coeff cneg on p, 'relu' -> coeff (cpos-cneg) on relu(p)."""
    terms = []
    for h, r in enumerate(routes):
        if r == "D":
            terms.append((h, "raw"))
            terms.append((h, "relu"))
    return terms


def _build_graph(n_jt, slow_mask, bv_nz, head_consts):
    """Build the SPMD Bass graph. head_consts: {'heads': [{'c1': cpos*8,
    'c2': cneg*8}, ...]}."""
    AF = mybir.ActivationFunctionType
    OP = mybir.AluOpType
    NJ = n_jt * 128  # padded key count

    nc = bacc.Bacc("TRN2", debug=False)

    pT_ext = nc.declare_dram_parameter("pT", [NJ, IH], BF16, isOutput=False)
    xT_ext = nc.declare_dram_parameter("xT", [E, N], BF16, isOutput=False)
    xTq_ext = nc.declare_dram_parameter("xTq", [E, IH], BF16, isOutput=False)
    wqT_ext = nc.declare_dram_parameter("wqT", [E, E], BF16, isOutput=False)
    wkT_ext = nc.declare_dram_parameter("wkT", [E, E], BF16, isOutput=False)
    wvT_ext = nc.declare_dram_parameter("wvT", [E, E], BF16, isOutput=False)
    woT_ext = nc.declare_dram_parameter("woT", [E, E], BF16, isOutput=False)
    bq_ext = nc.declare_dram_parameter("bq2", [128, 4], F32, isOutput=False)
    bk_ext = nc.declare_dram_parameter("bk2", [128, 4], F32, isOutput=False)
    bv_ext = nc.declare_dram_parameter("bv2", [64, 8], F32, isOutput=False)
    mv_ext = nc.declare_dram_parameter("mvec", [128, n_jt], F32, isOutput=False)
    terms = _idm_terms(ROUTES)
    n_t = len(terms)
    if n_t:
        idm_ext = nc.declare_dram_parameter(
            "idm", [128, n_t, 128], BF16, isOutput=False)
    out_ext = nc.declare_dram_parameter("out", [IH, E], F32, isOutput=True)

    # term index per (head, which)
    tidx = {key: i for i, key in enumerate(terms)}
    heads = head_consts["heads"]

    with tile.TileContext(nc) as tc:
        with (
            tc.tile_pool(name="persist", bufs=1) as pers,
        ):
            prol_cm = tc.tile_pool(name="prolog", bufs=1)
            prol = prol_cm.__enter__()
            psp_cm = tc.tile_pool(name="psum_p", bufs=2, space=bass.MemorySpace.PSUM)
            ps = psp_cm.__enter__()
            # ---------------- prologue: loads ----------------
            xT_sb = prol.tile([128, 4, N], BF16, tag="xT")
            xTq_sb = prol.tile([128, 4, IH], BF16, tag="xTq")
            wq_sb = prol.tile([128, 4, E], BF16, tag="wq")
            wk_sb = prol.tile([128, 4, E], BF16, tag="wk")
            wv_sb = prol.tile([128, 4, E], BF16, tag="wv")
            wo_sb = pers.tile([128, 4, E], BF16, tag="wo")
            bq_sb = pers.tile([128, 4], F32, tag="bq")
            bk_sb = pers.tile([128, 4], F32, tag="bk")
            bv_sb = pers.tile([64, 8], F32, tag="bv")
            mv_sb = pers.tile([128, n_jt], F32, tag="mv")
            p_sb = pers.tile([128, n_jt, IH], BF16, tag="p")
            if n_t:
                idm_sb = pers.tile([128, n_t, 128], BF16, tag="idm")
                pp_sb = pers.tile([128, n_jt, IH], BF16, tag="pp")

            for cc in range(4):
                nc.sync.dma_start(xTq_sb[:, cc, :], xTq_ext[cc * 128:(cc + 1) * 128, :])
                nc.sync.dma_start(wq_sb[:, cc, :], wqT_ext[cc * 128:(cc + 1) * 128, :])
            for cc in range(4):
                nc.sync.dma_start(xT_sb[:, cc, :], xT_ext[cc * 128:(cc + 1) * 128, :])
                nc.sync.dma_start(wk_sb[:, cc, :], wkT_ext[cc * 128:(cc + 1) * 128, :])
            for cc in range(4):
                nc.sync.dma_start(wv_sb[:, cc, :], wvT_ext[cc * 128:(cc + 1) * 128, :])
            nc.sync.dma_start(bq_sb[:], bq_ext[:])
            nc.sync.dma_start(bk_sb[:], bk_ext[:])
            nc.sync.dma_start(bv_sb[:], bv_ext[:])
            nc.sync.dma_start(mv_sb[:], mv_ext[:])
            if n_t:
                nc.sync.dma_start(idm_sb[:], idm_ext[:])
            for jt in range(3):
                nc.sync.dma_start(p_sb[:, jt, :], pT_ext[jt * 128:(jt + 1) * 128, :])
                if n_t:
                    nc.vector.tensor_scalar(
                        pp_sb[:, jt, :], p_sb[:, jt, :], 0.0, None, op0=OP.max)
            for cc in range(4):
                nc.sync.dma_start(wo_sb[:, cc, :], woT_ext[cc * 128:(cc + 1) * 128, :])
            for jt in range(3, n_jt):
                nc.sync.dma_start(p_sb[:, jt, :], pT_ext[jt * 128:(jt + 1) * 128, :])
                if n_t:
                    nc.vector.tensor_scalar(
                        pp_sb[:, jt, :], p_sb[:, jt, :], 0.0, None, op0=OP.max)

            # ---------------- projections ----------------
            # Q^T and K^T head-PAIRED: [128, free] with head 2et in
            # partitions 0-63, head 2et+1 in partitions 64-127.
            QT_sb = pers.tile([128, 4, IH], BF16, tag="QT")
            KT_0 = pers.tile([128, NJ], BF16, tag="KT0")
            KT_1 = pers.tile([128, NJ], BF16, tag="KT1")
            KT_2 = pers.tile([128, NJ], BF16, tag="KT2")
            KT_3 = pers.tile([128, NJ], BF16, tag="KT3")
            KTs = [KT_0, KT_1, KT_2, KT_3]
            V_sb = pers.tile([128, n_jt, H, 65], BF16, tag="V")

            for et in range(4):  # Q^T
                pq = ps.tile([128, IH], F32, tag="pq")
                for ib in range(2):
                    for cc in range(4):
                        nc.tensor.matmul(
                            pq[:, ib * 512:(ib + 1) * 512],
                            wq_sb[:, cc, et * 128:(et + 1) * 128],
                            xTq_sb[:, cc, ib * 512:(ib + 1) * 512],
                            start=(cc == 0), stop=(cc == 3),
                        )
                nc.scalar.activation(
                    QT_sb[:, et, :], pq[:],
                    AF.Identity, bias=bq_sb[:, et:et + 1],
                )

            jbs = []
            jpos = 0
            while jpos < NJ:
                jbs.append((jpos, min(512, NJ - jpos)))
                jpos += 512
            for et in range(4):  # K^T
                for (js, jl) in jbs:
                    pk = ps.tile([128, 512], F32, tag="pk")
                    for cc in range(4):
                        nc.tensor.matmul(
                            pk[:, 0:jl],
                            wk_sb[:, cc, et * 128:(et + 1) * 128],
                            xT_sb[:, cc, js:js + jl],
                            start=(cc == 0), stop=(cc == 3),
                        )
                    nc.scalar.activation(
                        KTs[et][:, js:js + jl], pk[:, 0:jl],
                        AF.Identity, bias=bk_sb[:, et:et + 1],
                    )

            for jt in range(n_jt):  # V (+ ones column)
                pv = ps.tile([128, 512], F32, tag="pv")
                for cc in range(4):
                    nc.tensor.matmul(
                        pv[:],
                        xT_sb[:, cc, jt * 128:(jt + 1) * 128],
                        wv_sb[:, cc, :],
                        start=(cc == 0), stop=(cc == 3),
                    )
                nc.vector.tensor_copy(V_sb[:, jt, :, 0:64], pv[:])
            nc.vector.memset(V_sb[:, :, :, 64:65], 1.0)
            psp_cm.__exit__(None, None, None)
            prol_cm.__exit__(None, None, None)
            work_cm = tc.tile_pool(name="work", bufs=3)
            work = work_cm.__enter__()
            workp_cm = tc.tile_pool(name="workp", bufs=7)
            workp = workp_cm.__enter__()
            psm_cm = tc.tile_pool(name="psum_m", bufs=4, space=bass.MemorySpace.PSUM)
            ps = psm_cm.__enter__()
            psm2_cm = tc.tile_pool(name="psum_m2", bufs=2, space=bass.MemorySpace.PSUM)
            ps2 = psm2_cm.__enter__()

            # ---------------- attention ----------------
            AO_sb = pers.tile([128, 4, IH], BF16, tag="AO")

            groups = []
            g0 = 0
            while g0 < n_jt:
                groups.append((g0, min(EXP_GROUP, n_jt - g0)))
                g0 += EXP_GROUP

            def emit_idm_mms(h, which, s_ps, jt, ib, start, stop):
                """Quadrant-packed diag-matmul pair adding a bias term into
                the score psum tile."""
                src = p_sb if which == "raw" else pp_sb
                t = tidx[(h, which)]
                sl = slice(ib * 512, (ib + 1) * 512)
                nc.tensor.matmul(
                    s_ps[0:64, :],
                    idm_sb[0:64, t, 0:64],
                    src[0:64, jt, sl],
                    start=start, stop=stop, skip_group_check=True,
                )
                nc.tensor.matmul(
                    s_ps[64:128, :],
                    idm_sb[64:128, t, 64:128],
                    src[64:128, jt, sl],
                    start=start, stop=stop, skip_group_check=True,
                )

            for et in range(4):
                hA, hB = 2 * et, 2 * et + 1
                P_groups = {hA: {}, hB: {}}
                X_cur = {}
                ao_t = {}
                for h in (hA, hB):
                    ao_t[h] = ps2.tile([128, 1024], F32, tag="ao",
                                       name=f"ao_{h}")
                for (gs, gl) in groups:
                    for h in (hA, hB):
                        P_groups[h][gs] = workp.tile(
                            [128, EXP_GROUP * IH], BF16, tag="P",
                            name=f"P_{h}_{gs}")
                        if ROUTES[h] != "D":
                            X_cur[h] = work.tile(
                                [128, EXP_GROUP * IH], BF16, tag="X",
                                name=f"X_{h}_{gs}")
                    for k in range(gl):
                        jt = gs + k
                        s_tiles = {}
                        for ib in range(2):
                            sA = ps.tile([128, 512], F32, tag="s")
                            sB = ps.tile([128, 512], F32, tag="s")
                            s_tiles[(hA, ib)] = sA
                            s_tiles[(hB, ib)] = sB
                            # paired K=64 score matmuls (concurrent row tiles)
                            for h, s_ps in ((hA, sA), (hB, sB)):
                                half = h % 2
                                psl = slice(half * 64, half * 64 + 64)
                                has_idm = ROUTES[h] == "D"
                                nc.tensor.matmul(
                                    s_ps[:],
                                    KTs[et][psl, jt * 128:(jt + 1) * 128],
                                    QT_sb[psl, et, ib * 512:(ib + 1) * 512],
                                    start=True, stop=not has_idm,
                                    skip_group_check=has_idm,
                                )
                            for h, s_ps in ((hA, sA), (hB, sB)):
                                if ROUTES[h] == "D":
                                    emit_idm_mms(h, "raw", s_ps, jt, ib,
                                                 False, False)
                                    emit_idm_mms(h, "relu", s_ps, jt, ib,
                                                 False, True)
                        # elementwise per head / ib
                        for h in (hA, hB):
                            hc = heads[h]
                            for ib in range(2):
                                s_ps = s_tiles[(h, ib)]
                                xo = k * IH + ib * 512
                                if ROUTES[h] == "D":
                                    nc.scalar.activation(
                                        P_groups[h][gs][:, xo:xo + 512],
                                        s_ps[:], AF.Exp,
                                        bias=(mv_sb[:, jt:jt + 1]
                                              if slow_mask else 0.0),
                                        scale=0.125,
                                    )
                                else:  # 'F': fused bias+add in one DVE op
                                    nc.vector._custom_dve(
                                        BIAS_ADD_OP,
                                        out=X_cur[h][:, xo:xo + 512],
                                        in0=p_sb[:, jt, ib * 512:(ib + 1) * 512],
                                        in1=s_ps[:],
                                        s0=hc["c2"], s1=hc["c1"] - hc["c2"],
                                    )
                    # grouped exp for non-D heads
                    for h in (hA, hB):
                        if ROUTES[h] == "D":
                            continue
                        X = X_cur[h]
                        P_g = P_groups[h][gs]
                        if not slow_mask:
                            nc.scalar.activation(
                                P_g[:, 0:gl * IH],
                                X[:, 0:gl * IH], AF.Exp, bias=0.0, scale=0.125,
                            )
                        else:
                            for k in range(gl):
                                jt = gs + k
                                nc.scalar.activation(
                                    P_g[:, k * IH:(k + 1) * IH],
                                    X[:, k * IH:(k + 1) * IH], AF.Exp,
                                    bias=mv_sb[:, jt:jt + 1], scale=0.125,
                                )
                    # attn @ V' for this group (denominator rides in row 64)
                    for h in (hA, hB):
                        for k in range(gl):
                            jc = gs + k
                            for ib in range(2):
                                nc.tensor.matmul(
                                    ao_t[h][0:65, ib * 512:(ib + 1) * 512],
                                    V_sb[:, jc, h, :],
                                    P_groups[h][gs][:, k * IH + ib * 512:
                                                    k * IH + (ib + 1) * 512],
                                    start=(jc == 0), stop=(jc == n_jt - 1),
                                )

                # normalize by the denominator row.  One ScalarE copy drains
                # the PSUM accumulator (frees the bank for the next pair's
                # attn@V immediately) and doubles as the z-row extraction.
                for h in (hA, hB):
                    ao = ao_t[h]
                    aob = work.tile([64, 1024], F32, tag="aob", bufs=2)
                    nc.scalar.copy(aob[:], ao[0:64, :])
                    z_sb = work.tile([1, 1024], F32, tag="z", bufs=2)
                    nc.scalar.copy(z_sb[:], ao[64:65, :])
                    rz = work.tile([1, 1024], F32, tag="rz", bufs=2)
                    nc.vector.reciprocal_approx_fast(rz[:], z_sb[:])
                    rz_b = work.tile([64, 1024], F32, tag="rzb", bufs=2)
                    nc.gpsimd.partition_broadcast(rz_b[:], rz[:])
                    aosl = AO_sb[(h % 2) * 64:(h % 2) * 64 + 64, h // 2, :]
                    if not bv_nz:
                        nc.vector.tensor_tensor(
                            aosl, aob[:], rz_b[:], op=OP.mult)
                    else:
                        t_t = work.tile([64, 1024], F32, tag="aot")
                        nc.vector.tensor_tensor(
                            t_t[:], aob[:], rz_b[:], op=OP.mult)
                        nc.vector.tensor_scalar(
                            aosl, t_t[:], bv_sb[:, h:h + 1], None, op0=OP.add)

            # ---------------- output projection ----------------
            for ic in range(8):
                po = ps.tile([128, 512], F32, tag="s")
                for cc in range(4):
                    nc.tensor.matmul(
                        po[:],
                        AO_sb[:, cc, ic * 128:(ic + 1) * 128],
                        wo_sb[:, cc, :],
                        start=(cc == 0), stop=(cc == 3),
                    )
                o_sb = work.tile([128, E], F32, tag="osb")
                nc.scalar.copy(o_sb[:], po[:])
                nc.sync.dma_start(out_ext[ic * 128:(ic + 1) * 128, :], o_sb[:])
            psm2_cm.__exit__(None, None, None)
            psm_cm.__exit__(None, None, None)
            workp_cm.__exit__(None, None, None)
            work_cm.__exit__(None, None, None)

    nc.compile()
    return nc


_GRAPH_CACHE = {}
_LAST_IN_MAPS = None


def _numpy_reference(x, pairwise_feats, key_padding_mask, Wq, bq, Wk, bk, Wv, bv,
                     Wo, bo, W1, b1, W2, b2):
    """Pure-numpy fallback (only used if assumptions are violated)."""
    def proj(W, b):
        return (x @ W.T + b).reshape(B, N, H, D).transpose(0, 2, 1, 3)
    Q, K, V = proj(Wq, bq), proj(Wk, bk), proj(Wv, bv)
    scores = np.einsum("bhnd,bhmd->bhnm", Q, K) / np.sqrt(D)
    h = np.maximum(pairwise_feats @ W1.T + b1, 0.0)
    bias = (h @ W2.T + b2).transpose(0, 3, 1, 2)
    scores = scores + bias
    scores = np.where(key_padding_mask[:, None, None, :], -np.inf, scores)
    scores = scores - scores.max(axis=-1, keepdims=True)
    e = np.exp(scores)
    attn = e / e.sum(axis=-1, keepdims=True)
    out = np.einsum("bhnm,bhmd->bhnd", attn, V)
    out = out.transpose(0, 2, 1, 3).reshape(B, N, E)
    return out @ Wo.T + bo


def kernel(**inputs):
    inp = {k: np.asarray(v) for k, v in inputs.items()}
    x = inp["x"].astype(np.float32)
    pw = inp["pairwise_feats"].astype(np.float32)
    mask = inp["key_padding_mask"].astype(bool)
    Wq, bq = inp["Wq"].astype(np.float32), inp["bq"].astype(np.float32)
    Wk, bk = inp["Wk"].astype(np.float32), inp["bk"].astype(np.float32)
    Wv, bv = inp["Wv"].astype(np.float32), inp["bv"].astype(np.float32)
    Wo, bo = inp["Wo"].astype(np.float32), inp["bo"].astype(np.float32)
    W1, b1 = inp["W1"].astype(np.float32), inp["b1"].astype(np.float32)
    W2, b2 = inp["W2"].astype(np.float32), inp["b2"].astype(np.float32)

    if not (np.all(b1 == 0.0) and x.shape == (B, N, E)
            and pw.shape == (B, N, N, 1) and W1.shape[1] == 1):
        return _numpy_reference(x, pw, mask, Wq, bq, Wk, bk, Wv, bv, Wo, bo,
                                W1, b1, W2, b2).astype(np.float32)

    # per-head piecewise-linear bias coefficients (b1 == 0; b2 drops out of
    # softmax as a per-head constant shift)
    w1 = W1[:, 0]
    cpos = (W2 * np.maximum(w1, 0.0)[None, :]).sum(axis=1) * 8.0  # pre-scaled by sqrt(D)
    cneg = (W2 * np.minimum(w1, 0.0)[None, :]).sum(axis=1) * 8.0

    heads = [dict(c1=float(cpos[h]), c2=float(cneg[h])) for h in range(H)]

    # mask handling (per batch; n_jt must be uniform across cores = max)
    valid = ~mask  # [B, N]
    n_valid = np.array([int(np.nonzero(valid[b])[0].max()) + 1 for b in range(B)])
    n_jt = int(np.ceil(n_valid.max() / 128))
    NJ = n_jt * 128
    mvecs = np.where(mask[:, :NJ], NEG_BIG, 0.0).astype(np.float32)  # [B, NJ]
    slow_mask = bool(mvecs.any())
    bv_nz = bool(np.any(bv != 0.0))

    # host-side preprocessing shared across cores
    bf = ml_dtypes.bfloat16
    terms = _idm_terms(ROUTES)
    if terms:
        idm = np.zeros((128, len(terms), 128), np.float32)
        eye = np.eye(128, dtype=np.float32)
        for t, (h, which) in enumerate(terms):
            c = float(cneg[h]) if which == "raw" else float(cpos[h] - cneg[h])
            idm[:, t, :] = eye * c
        idm = idm.astype(bf)
    wqT = np.ascontiguousarray(Wq.T).astype(bf)
    wkT = np.ascontiguousarray(Wk.T).astype(bf)
    wvT = np.ascontiguousarray(Wv.T).astype(bf)
    woT = np.ascontiguousarray(Wo.T).astype(bf)
    bq2 = np.ascontiguousarray(bq.reshape(4, 128).T).astype(np.float32)
    bk2 = np.ascontiguousarray(bk.reshape(4, 128).T).astype(np.float32)
    bv2 = np.ascontiguousarray(bv.reshape(8, 64).T).astype(np.float32)

    in_maps = []
    for c in range(NCORES):
        b, half = c // 2, c % 2
        i0 = half * IH
        xTb = np.ascontiguousarray(x[b].T).astype(bf)
        pT = np.ascontiguousarray(pw[b, :, :NJ, 0].T[:, i0:i0 + IH]).astype(bf)
        xTq = np.ascontiguousarray(xTb[:, i0:i0 + IH])
        mv2 = np.ascontiguousarray(mvecs[b].reshape(n_jt, 128).T).astype(np.float32)
        im = {
            "pT": pT, "xT": xTb, "xTq": xTq, "wqT": wqT, "wkT": wkT,
            "wvT": wvT, "woT": woT, "bq2": bq2, "bk2": bk2, "bv2": bv2,
            "mvec": mv2,
        }
        if terms:
            im["idm"] = idm
        in_maps.append(im)

    key = (n_jt, slow_mask, bv_nz, ROUTES, EXP_GROUP,
           tuple(np.round(cpos, 10)), tuple(np.round(cneg, 10)))
    if key not in _GRAPH_CACHE:
        _GRAPH_CACHE[key] = _build_graph(n_jt, slow_mask, bv_nz, {"heads": heads})
    nc = _GRAPH_CACHE[key]
    global _LAST_IN_MAPS
    _LAST_IN_MAPS = in_maps
    res = run_bass_kernel_spmd(nc, in_maps, core_ids=list(range(NCORES)))

    out = np.empty((B, N, E), np.float32)
    for c in range(NCORES):
        b, half = c // 2, c % 2
        out[b, half * IH:(half + 1) * IH, :] = res.results[c]["out"]
    return out + bo[None, None, :]
